# revision 1
# baseline (speedup 1.0000x reference)
"""Trainium2 Bass kernel for nn_Attention_Encoder (conv1x1 -> time-softmax attention -> relu-GRU).

Sharding: pure data parallelism. 1024 segments split across 8 cores (S=128
per core); weights replicated. v2 redesign vs baseline:

phase A (per segment pair):
  x_T [C,2,T] via transpose-DMA; conv_T = relu(Wc^T x_T) (DVE TSP evac);
  conv_N obtained by PE transposes of the relu'd conv_T (bf16 PSUM
  pass-through, evacuated by a 2x-rate TensorCopy); scores -> exp (ACT,
  fused row-sum); x_att = E*conv on Pool (TT), then *rinv on DVE (4x TSP),
  stored [128, k, S, T] so writes are packed.

phase B: two software-pipelined chains of SC=64 segments (B half a step
  behind A) hide the recurrence latency. Per chain-step: one PSUM tile
  [128, 8, SC] holds z,r,rh,xh (concurrent per-bank accumulation groups,
  sim check skipped -- hardware zeroes only written bytes); ACT evacuates
  xh and runs one combined sigmoid over [z;r]; DVE: t1=r*rh(PSUM),
  q=t1+xh, hh=relu(q) (4x TSP); Pool: d=h-hh, e=z*d, h'=hh+e.
  GRU biases (generic path) are added via rank-1 matmuls into PSUM.
"""

import contextlib
import os
import sys

sys.path.insert(0, "/opt/trn_rl_repo")

import numpy as np
import ml_dtypes

import concourse.bass as bass
import concourse.tile as tile
from concourse import mybir
from concourse.bass_utils import run_bass_kernel_spmd

F32 = mybir.dt.float32
BF16 = mybir.dt.bfloat16
AF = mybir.ActivationFunctionType
OP = mybir.AluOpType

B, LTMS, TTS, C_IN, FF, HH = 64, 16, 256, 128, 256, 256
NCORES = 8
S = (B * LTMS) // NCORES  # 128 segments per core
T = TTS                   # 256 timesteps
SC = S // 2               # 64 segments per chain

# bfpack column layout (bf16): conv_w | attn_w | gru_w | gru_u | identity
BP_CW = 0
BP_AW = BP_CW + FF              # 256
BP_WG = BP_AW + 2 * T           # 768
BP_WU = BP_WG + 2 * 3 * HH      # 2304
BP_ID = BP_WU + 2 * 3 * HH      # 3840
BP_W = BP_ID + 128              # 3968


def build(zero_bias: bool) -> bass.Bass:
    nc = bass.Bass("TRN2", target_bir_lowering=False)

    x_d = nc.dram_tensor("x_shard", [S, T, C_IN], BF16, kind="ExternalInput")
    bp_d = nc.dram_tensor("bfpack", [128, BP_W], BF16, kind="ExternalInput")
    if not zero_bias:
        cb_d = nc.dram_tensor("conv_b2", [128, 2], F32, kind="ExternalInput")
        ab_d = nc.dram_tensor("attn_b", [1, T], BF16, kind="ExternalInput")
        # gru bias rows for rank-1 PSUM adds: [1, 8*128] bf16
        # order: z0 z1 r0 r1 (bi+br) | rh0 rh1 (br_h) | xh0 xh1 (bi_h)
        gb_d = nc.dram_tensor("gbias_row", [1, 8 * 128], BF16,
                              kind="ExternalInput")
    out_d = nc.dram_tensor("h_out", [S, HH], F32, kind="ExternalOutput")

    with tile.TileContext(nc, trace_sim=bool(os.environ.get("KTRACE"))) as tc:
        with contextlib.ExitStack() as ctx:
            singles = ctx.enter_context(tc.tile_pool(name="singles", bufs=1))

            bp_sb = singles.tile([128, BP_W], BF16)
            nc.sync.dma_start(bp_sb, bp_d[:])

            cw_sb = bp_sb[:, BP_CW:BP_CW + FF]
            aw_sb = bp_sb[:, BP_AW:BP_AW + 2 * T].rearrange(
                "p (k n) -> p k n", k=2)
            wg_sb = bp_sb[:, BP_WG:BP_WG + 1536].rearrange(
                "p (k n) -> p k n", k=2)
            wu_sb = bp_sb[:, BP_WU:BP_WU + 1536].rearrange(
                "p (k n) -> p k n", k=2)
            ident_bf = bp_sb[:, BP_ID:BP_ID + 128]

            # global x_att store: [F%128, F-chunk, S, T] bf16 (T packed)
            xatt = singles.tile([128, 2, S, T], BF16)

            if not zero_bias:
                cb_sb = singles.tile([128, 2], F32)
                nc.sync.dma_start(cb_sb, cb_d[:])
                ab_row = singles.tile([1, T], BF16)
                nc.sync.dma_start(ab_row, ab_d[:])
                gb_row = singles.tile([1, 8 * 128], BF16)
                nc.sync.dma_start(gb_row, gb_d[:])
                ones_col = singles.tile([1, 128], BF16)
                nc.vector.memset(ones_col, 1.0)
                ones_sc = ones_col[:, :SC]

            # ---------------- phase A ----------------
            apool = ctx.enter_context(tc.tile_pool(name="apool", bufs=4))
            with contextlib.ExitStack() as actx:
                apsum = actx.enter_context(
                    tc.tile_pool(name="apsum", bufs=1, space="PSUM"))

                # PE warmup: consume the weight-pack DMA on PE early
                # (borrows the ps_cn tag so phase A stays within 8 PSUM banks)
                ps_w1 = apsum.tile([128, 128], BF16, tag="ps_cn", bufs=2)
                nc.tensor.transpose(ps_w1, ident_bf, ident_bf)

                for s2 in range(S // 2):
                    s = 2 * s2
                    x_t = apool.tile([128, 2, T], BF16, tag="x_t", bufs=6)
                    nc.sync.dma_start_transpose(x_t[:, 0, :], x_d[s])
                    nc.sync.dma_start_transpose(x_t[:, 1, :], x_d[s + 1])

                    # conv_T = relu(W_c^T @ x_T): [F(2ch), seg, T]
                    ps_ct = apsum.tile([128, 2, 2, T], F32, tag="ps_ct",
                                       bufs=1)
                    for m in range(2):
                        nc.tensor.matmul(
                            ps_ct[:, m, :, :], cw_sb[:, bass.ts(m, 128)],
                            x_t, start=True, stop=True)
                    conv_t = apool.tile([128, 2, 2, T], BF16, tag="conv_t")
                    if zero_bias:
                        # single wide evac+relu amortizes the PSUM access
                        nc.vector.tensor_scalar_max(conv_t, ps_ct, 0.0)
                    else:
                        for mc in range(2):
                            nc.vector.tensor_scalar(
                                conv_t[:, mc, :, :], ps_ct[:, mc, :, :],
                                cb_sb[:, mc:mc + 1], 0.0, OP.add, OP.max)

                    # conv_N via PE transposes of relu'd conv_T (bf16 psum)
                    ps_cn = apsum.tile([128, 2, 2, FF], BF16, tag="ps_cn",
                                       bufs=2)
                    for seg in range(2):
                        for tch in range(2):
                            for m in range(2):
                                nc.tensor.transpose(
                                    ps_cn[:, seg, tch, bass.ts(m, 128)],
                                    conv_t[:, m, seg, bass.ts(tch, 128)],
                                    ident_bf)
                    conv_n = apool.tile([128, 2, 2, FF], BF16, tag="conv_n")
                    nc.vector.tensor_copy(conv_n, ps_cn)

                    # scores = conv_N^T @ A (+ b): [seg, F-ch, T]
                    ps_s = apsum.tile([128, 2, 2, T], F32, tag="ps_s", bufs=2)
                    for seg in range(2):
                        for m in range(2):
                            for k in range(2):
                                nc.tensor.matmul(
                                    ps_s[:, seg, m, :],
                                    conv_n[:, seg, k, bass.ts(m, 128)],
                                    aw_sb[:, k, :],
                                    start=(k == 0),
                                    stop=(k == 1) and zero_bias)
                            if not zero_bias:
                                nc.tensor.matmul(
                                    ps_s[:, seg, m, :], ones_col, ab_row,
                                    start=False, stop=True)

                    ee = apool.tile([128, 2, 2, T], BF16, tag="ee")
                    esum = apool.tile([128, 4], F32, tag="esum")
                    es4 = esum.rearrange("p (a b) -> p a b", a=2)
                    for seg in range(2):
                        for m in range(2):
                            nc.scalar.activation(
                                ee[:, seg, m, :], ps_s[:, seg, m, :], AF.Exp,
                                accum_out=es4[:, seg, m:m + 1])
                    rinv = apool.tile([128, 4], F32, tag="rinv")
                    nc.vector.reciprocal(rinv, esum)
                    ri4 = rinv.rearrange("p (a b) -> p a b", a=2)

                    # x_att[:, m, s+seg, :] = E * rinv * conv_T  (packed T)
                    # split: ec = E*conv on Pool (TT), then *rinv on DVE (4x)
                    ec = apool.tile([128, 2, 2, T], BF16, tag="ec")
                    for seg in range(2):
                        nc.gpsimd.tensor_mul(
                            ec[:, seg, :, :], ee[:, seg, :, :],
                            conv_t[:, :, seg, :])
                    for seg in range(2):
                        for m in range(2):
                            nc.gpsimd.tensor_mul(
                                xatt[:, m, s + seg, :], ec[:, seg, m, :],
                                ri4[:, seg, m:m + 1].broadcast_to([128, T]))

            # ---------------- phase B: GRU over T steps, 2 chains ----------
            # gate columns in W/U: z=[0,256) m0,1 ; r=[256,512) m2,3 ;
            # h=[512,768) m4,5
            # psum tile layout [128, 8, SC]: z0 z1 r0 r1 | rh0 rh1 | xh0 xh1
            with contextlib.ExitStack() as bctx:
                hpool = bctx.enter_context(tc.tile_pool(name="hpool", bufs=2))
                gpool = bctx.enter_context(tc.tile_pool(name="gpool", bufs=3))
                bpsum = bctx.enter_context(
                    tc.tile_pool(name="bpsum", bufs=1, space="PSUM"))

                h_prev = [None, None]
                pend = [None, None]  # (t, ps, rz, xh_sb) awaiting elementwise

                def emit_pe_act(c, t):
                    """Matmuls + sigmoid + xh evac for (chain c, step t)."""
                    cb = c * SC
                    ps = bpsum.tile([128, 8, SC], F32, tag=f"ps{c}", bufs=3,
                                    name=f"ps{c}")
                    hp = h_prev[c]

                    # x-part matmuls (independent of h)
                    zr_stop = (t == 0) and zero_bias
                    for j, m in enumerate((0, 1)):      # z gates
                        for k in range(2):
                            nc.tensor.matmul(
                                ps[:, j, :], wg_sb[:, k, bass.ts(m, 128)],
                                xatt[:, k, cb:cb + SC, t],
                                start=(k == 0), stop=(k == 1) and zr_stop,
                                skip_group_check=True)
                    for j, m in enumerate((2, 3)):      # r gates
                        for k in range(2):
                            nc.tensor.matmul(
                                ps[:, 2 + j, :],
                                wg_sb[:, k, bass.ts(m, 128)],
                                xatt[:, k, cb:cb + SC, t],
                                start=(k == 0), stop=(k == 1) and zr_stop,
                                skip_group_check=True)
                    for j, m in enumerate((4, 5)):      # h gate (xh)
                        for k in range(2):
                            nc.tensor.matmul(
                                ps[:, 6 + j, :],
                                wg_sb[:, k, bass.ts(m, 128)],
                                xatt[:, k, cb:cb + SC, t],
                                start=(k == 0),
                                stop=(k == 1) and zero_bias,
                                skip_group_check=True)

                    if not zero_bias:
                        # rank-1 bias adds; z0..r1 into [0:4],
                        # xh into [6:8], rh (br_h) into [4:6]
                        for j in range(4):
                            nc.tensor.matmul(
                                ps[:, j, :], gb_row[:, bass.ts(j, 128)],
                                ones_sc, start=False, stop=(t == 0),
                                skip_group_check=True)
                        for j in range(2):
                            nc.tensor.matmul(
                                ps[:, 6 + j, :],
                                gb_row[:, bass.ts(6 + j, 128)],
                                ones_sc, start=False, stop=True,
                                skip_group_check=True)
                        for j in range(2):
                            nc.tensor.matmul(
                                ps[:, 4 + j, :],
                                gb_row[:, bass.ts(4 + j, 128)],
                                ones_sc, start=True, stop=(t == 0),
                                skip_group_check=True)

                    # ACT evacuates xh early (depends only on Wx)
                    xh_sb = gpool.tile([128, 2, SC], BF16, tag=f"xh{c}",
                                       bufs=2, name=f"xh{c}")
                    nc.scalar.copy(xh_sb, ps[:, 6:8, :])

                    rz = gpool.tile([128, 4, SC], BF16, tag=f"rz{c}",
                                    bufs=2, name=f"rz{c}")
                    if t > 0:
                        # U-part: r,z first (gate the sigmoid), then rh
                        for j, m in enumerate((2, 3)):
                            for k in range(2):
                                nc.tensor.matmul(
                                    ps[:, 2 + j, :],
                                    wu_sb[:, k, bass.ts(m, 128)],
                                    hp[:, k, :],
                                    start=False, stop=(k == 1),
                                    skip_group_check=True)
                        for j, m in enumerate((0, 1)):
                            for k in range(2):
                                nc.tensor.matmul(
                                    ps[:, j, :],
                                    wu_sb[:, k, bass.ts(m, 128)],
                                    hp[:, k, :],
                                    start=False, stop=(k == 1),
                                    skip_group_check=True)
                        for j, m in enumerate((4, 5)):  # rh
                            for k in range(2):
                                nc.tensor.matmul(
                                    ps[:, 4 + j, :],
                                    wu_sb[:, k, bass.ts(m, 128)],
                                    hp[:, k, :],
                                    start=(k == 0) and zero_bias,
                                    stop=(k == 1),
                                    skip_group_check=True)
                    # sigmoid over [z;r] in one ACT op
                    nc.scalar.activation(rz, ps[:, 0:4, :], AF.Sigmoid)
                    pend[c] = (t, ps, rz, xh_sb)

                def emit_dve(c):
                    """Elementwise chain for the pending (chain c) step.
                    The d/e blend ops run on Pool to cut DVE occupancy."""
                    t, ps, rz, xh_sb = pend[c]
                    hp = h_prev[c]
                    h_new = hpool.tile([128, 2, SC], BF16, tag=f"h{c}",
                                       name=f"h{c}")
                    hh = gpool.tile([128, 2, SC], BF16, tag=f"hh{c}",
                                    bufs=2, name=f"hh{c}")
                    d = gpool.tile([128, 2, SC], BF16, tag=f"d{c}",
                                   bufs=2, name=f"d{c}")
                    e = gpool.tile([128, 2, SC], BF16, tag=f"e{c}",
                                   bufs=2, name=f"e{c}")
                    have_rh = (t > 0) or not zero_bias
                    # rz[:,0:2] is already w = 1-z (z weights negated on
                    # host). Off-chain on Pool: m1 = z*h = h - w*h
                    w = rz[:, 0:2, :]
                    if t > 0:
                        mw = gpool.tile([128, 2, SC], BF16, tag=f"mw{c}",
                                        bufs=2, name=f"mw{c}")
                        m1 = gpool.tile([128, 2, SC], BF16, tag=f"m1{c}",
                                        bufs=2, name=f"m1{c}")
                        nc.gpsimd.tensor_mul(mw, w, hp)
                        nc.gpsimd.tensor_sub(m1, hp, mw)
                    if have_rh:
                        t1 = gpool.tile([128, 2, SC], BF16, tag=f"t1{c}",
                                        bufs=2, name=f"t1{c}")
                        q = gpool.tile([128, 2, SC], BF16, tag=f"q{c}",
                                       bufs=2, name=f"q{c}")
                        nc.vector.tensor_mul(t1, rz[:, 2:4, :], ps[:, 4:6, :])
                        nc.vector.tensor_add(q, t1, xh_sb)
                    else:
                        q = xh_sb
                    # hh = relu(q) (TSP, 4x); then only 2 Pool ops on-chain:
                    # h' = w*hh + m1
                    nc.vector.tensor_scalar_max(hh, q, 0.0)
                    if t > 0:
                        nc.gpsimd.tensor_mul(e, w, hh)         # e = (1-z)*hh
                        nc.gpsimd.tensor_add(h_new, e, m1)
                    else:
                        nc.gpsimd.tensor_mul(h_new, w, hh)     # h0 = 0
                    h_prev[c] = h_new

                # software-pipelined: chain B runs half a step behind A.
                # Absolute-time pins phase-lock the two chains: a pin that's
                # already past is a no-op, so transient overruns self-correct.
                pin_base = float(os.environ.get("PIN_BASE", "228000"))  # inert unless PIN_P>0
                pin_p = float(os.environ.get("PIN_P", "0"))
                for t in range(T):
                    tp = pin_base + t * pin_p
                    with tc.tile_wait_until(tp / 1e6, enable=pin_p > 0):
                        emit_pe_act(0, t)
                    if t > 0:
                        with tc.tile_wait_until(tp / 1e6,
                                                enable=pin_p > 0):
                            emit_dve(1)
                    with tc.tile_wait_until((tp + 0.5 * pin_p) / 1e6,
                                            enable=pin_p > 0):
                        emit_pe_act(1, t)
                    with tc.tile_wait_until((tp + 0.46 * pin_p) / 1e6,
                                            enable=pin_p > 0):
                        emit_dve(0)
                emit_dve(1)

                # output: transpose h back to [S, H] and store fp32
                ps_o = bpsum.tile([64, 2, 2, 128], BF16, tag="ps_o", bufs=1)
                for c in range(2):
                    for ch in range(2):
                        nc.tensor.transpose(
                            ps_o[:, c, ch, :], h_prev[c][:, ch, :], ident_bf)
                out_sb = gpool.tile([64, 2, 2, 128], F32, tag="out_sb")
                nc.vector.tensor_copy(out_sb, ps_o)
                for c in range(2):
                    nc.sync.dma_start(
                        out_d[c * SC:(c + 1) * SC].rearrange(
                            "s (ch p) -> s ch p", ch=2), out_sb[:, c])

    _split_multi_waits(nc)
    return nc


def _split_multi_waits(nc: bass.Bass):
    """Encode at most ONE semaphore wait per ISA instruction: hoist extras
    onto preceding same-engine NoOp carriers."""
    fn = nc.m.functions[0]
    for blk in fn.blocks:
        insts = list(blk.instructions)
        out = []
        changed = False
        for inst in insts:
            si = inst.sync_info
            waits = list(si.on_wait) if si is not None else []
            if len(waits) > 1:
                changed = True
                for w in waits[:-1]:
                    out.append(mybir.InstNoOp(
                        name=f"I-wsplit-{nc.next_id()}",
                        engine=inst.engine,
                        ins=[], outs=[],
                        sync_info=mybir.SyncInfo(on_wait=[w], on_update=[]),
                    ))
                inst.sync_info = mybir.SyncInfo(
                    on_wait=[waits[-1]], on_update=list(si.on_update))
            out.append(inst)
        if changed:
            blk.instructions = out


_CACHE = {}


def _get_nc(zero_bias: bool) -> bass.Bass:
    if zero_bias not in _CACHE:
        _CACHE[zero_bias] = build(zero_bias)
    return _CACHE[zero_bias]


def _pack_weights(conv_w, attn_w, gru_w, gru_u):
    bf = ml_dtypes.bfloat16
    # z-gate columns negated: sigmoid of the negated preact yields w = 1-z
    gru_w = gru_w.copy(); gru_w[:, :256] *= -1.0
    gru_u = gru_u.copy(); gru_u[:, :256] *= -1.0
    cw = (conv_w[0] if conv_w.ndim == 3 else conv_w).astype(bf)  # [128, 256]
    aw = attn_w.astype(bf).reshape(2, 128, T).transpose(1, 0, 2).reshape(
        128, 2 * T)
    wg = gru_w.astype(bf).reshape(2, 128, 768).transpose(1, 0, 2).reshape(
        128, 1536)
    wu = gru_u.astype(bf).reshape(2, 128, 768).transpose(1, 0, 2).reshape(
        128, 1536)
    ident = np.eye(128, dtype=np.float32).astype(bf)
    return np.ascontiguousarray(
        np.concatenate([cw, aw, wg, wu, ident], axis=1), bf)


def kernel(x, conv_w, conv_b, attn_w, attn_b, gru_w, gru_u, gru_b):
    x = np.asarray(x, dtype=np.float32)
    conv_w = np.asarray(conv_w, dtype=np.float32)
    conv_b = np.asarray(conv_b, dtype=np.float32)
    attn_w = np.asarray(attn_w, dtype=np.float32)
    attn_b = np.asarray(attn_b, dtype=np.float32)
    gru_w = np.asarray(gru_w, dtype=np.float32)
    gru_u = np.asarray(gru_u, dtype=np.float32)
    gru_b = np.asarray(gru_b, dtype=np.float32)

    zero_bias = (
        not conv_b.any() and not attn_b.any() and not gru_b.any())

    nc = _get_nc(zero_bias)

    xs_bf = x.reshape(B * LTMS, T, C_IN).astype(ml_dtypes.bfloat16)
    bfpack = _pack_weights(conv_w, attn_w, gru_w, gru_u)

    in_maps = []
    for c in range(NCORES):
        m = {
            "x_shard": np.ascontiguousarray(xs_bf[c * S: (c + 1) * S]),
            "bfpack": bfpack,
        }
        if not zero_bias:
            bi, br = gru_b[0], gru_b[1]
            comb = bi + br
            gbr = np.zeros((1, 8 * 128), np.float32)
            gbr[0, 0:512] = comb[0:512]          # z0 z1 r0 r1
            gbr[0, 0:256] *= -1.0                # negated z preact -> w
            gbr[0, 512:768] = br[512:768]        # rh0 rh1
            gbr[0, 768:1024] = bi[512:768]       # xh0 xh1
            m["conv_b2"] = np.ascontiguousarray(
                conv_b.reshape(2, 128).T, np.float32)
            m["attn_b"] = attn_b.reshape(1, T).astype(ml_dtypes.bfloat16)
            m["gbias_row"] = gbr.astype(ml_dtypes.bfloat16)
        in_maps.append(m)

    res = run_bass_kernel_spmd(nc, in_maps, core_ids=list(range(NCORES)))
    outs = [res.results[c]["h_out"] for c in range(NCORES)]
    h = np.concatenate(outs, axis=0)  # [1024, 256]
    return h.reshape(B, LTMS, HH).astype(np.float32)


if __name__ == "__main__":
    nc = _get_nc(True)
    print("built ok")



# revision 58
# speedup vs baseline: 1.1570x; 1.1570x over previous
"""Trainium2 Bass kernel for nn_Attention_Encoder (conv1x1 -> time-softmax attention -> relu-GRU).

Sharding: pure data parallelism. 1024 segments split across 8 cores (S=128
per core); weights replicated. v2 redesign vs baseline:

phase A (per segment pair):
  x_T [C,2,T] via transpose-DMA; conv_T = relu(Wc^T x_T) (DVE TSP evac);
  conv_N via SBUF->SBUF transpose-DMA (DMAT=1 default: frees PE and the
  DVE PSUM-evac copy; the 16 DMA engines are otherwise idle in phase A);
  scores -> exp (ACT, fused row-sum); x_att = E*conv on Pool (TT), then
  *rinv on DVE (4x TSP), stored [128, k, S, T] so writes are packed.

phase B: two software-pipelined chains of SC=64 segments (B half a step
  behind A) hide the recurrence latency. Per chain-step: one PSUM tile
  [128, 8, SC] holds z,r,rh,xh (concurrent per-bank accumulation groups,
  sim check skipped -- hardware zeroes only written bytes); ACT evacuates
  xh and runs one combined sigmoid over [z;r]; DVE: t1=r*rh(PSUM),
  q=t1+xh, then e=relu(q)*w in ONE fused scalar_tensor_tensor op (the
  r-path terminus); Pool (off the critical path): mw=w*h, m1=h-mw,
  h'=e+m1. Key latency trick (SPLIT_U): since h(t-1)=m1(t-1)+e(t-1),
  the U_r/U_z matmuls split by linearity into U*m1 (operand lands
  ~500ns early, runs off-path) + U*e, so each step's sigmoid triggers
  off e(t-1) instead of the later h'(t-1). Steady-state period
  ~1.82us/step vs 2.15us for the unsplit loop.
  GRU biases (generic path) are added via rank-1 matmuls into PSUM.
"""

import contextlib
import os
import sys

sys.path.insert(0, "/opt/trn_rl_repo")

import numpy as np
import ml_dtypes

import concourse.bass as bass
import concourse.tile as tile
from concourse import mybir
from concourse.bass_utils import run_bass_kernel_spmd

F32 = mybir.dt.float32
BF16 = mybir.dt.bfloat16
AF = mybir.ActivationFunctionType
OP = mybir.AluOpType

B, LTMS, TTS, C_IN, FF, HH = 64, 16, 256, 128, 256, 256
NCORES = 8
S = (B * LTMS) // NCORES  # 128 segments per core
T = TTS                   # 256 timesteps
SC = S // 2               # 64 segments per chain

# bfpack column layout (bf16): conv_w | attn_w | gru_w | gru_u | identity
BP_CW = 0
BP_AW = BP_CW + FF              # 256
BP_WG = BP_AW + 2 * T           # 768
BP_WU = BP_WG + 2 * 3 * HH      # 2304
BP_ID = BP_WU + 2 * 3 * HH      # 3840
BP_W = BP_ID + 128              # 3968


def build(zero_bias: bool) -> bass.Bass:
    nc = bass.Bass("TRN2", target_bir_lowering=False)

    x_d = nc.dram_tensor("x_shard", [S, T, C_IN], BF16, kind="ExternalInput")
    bp_d = nc.dram_tensor("bfpack", [128, BP_W], BF16, kind="ExternalInput")
    if not zero_bias:
        cb_d = nc.dram_tensor("conv_b2", [128, 2], F32, kind="ExternalInput")
        ab_d = nc.dram_tensor("attn_b", [1, T], BF16, kind="ExternalInput")
        # gru bias rows for rank-1 PSUM adds: [1, 8*128] bf16
        # order: z0 z1 r0 r1 (bi+br) | rh0 rh1 (br_h) | xh0 xh1 (bi_h)
        gb_d = nc.dram_tensor("gbias_row", [1, 8 * 128], BF16,
                              kind="ExternalInput")
    out_d = nc.dram_tensor("h_out", [S, HH], F32, kind="ExternalOutput")

    with tile.TileContext(nc, trace_sim=bool(os.environ.get("KTRACE"))) as tc:
        with contextlib.ExitStack() as ctx:
            singles = ctx.enter_context(tc.tile_pool(name="singles", bufs=1))

            bp_sb = singles.tile([128, BP_W], BF16)
            nc.sync.dma_start(bp_sb, bp_d[:])

            cw_sb = bp_sb[:, BP_CW:BP_CW + FF]
            aw_sb = bp_sb[:, BP_AW:BP_AW + 2 * T].rearrange(
                "p (k n) -> p k n", k=2)
            wg_sb = bp_sb[:, BP_WG:BP_WG + 1536].rearrange(
                "p (k n) -> p k n", k=2)
            wu_sb = bp_sb[:, BP_WU:BP_WU + 1536].rearrange(
                "p (k n) -> p k n", k=2)
            ident_bf = bp_sb[:, BP_ID:BP_ID + 128]

            # global x_att store: [F%128, F-chunk, S, T] bf16 (T packed)
            xatt = singles.tile([128, 2, S, T], BF16)

            # zero column for the zero-contribution stagger gate matmuls
            zero_col = singles.tile([128, 1], BF16)
            nc.vector.memset(zero_col, 0.0)

            if not zero_bias:
                cb_sb = singles.tile([128, 2], F32)
                nc.sync.dma_start(cb_sb, cb_d[:])
                ab_row = singles.tile([1, T], BF16)
                nc.sync.dma_start(ab_row, ab_d[:])
                gb_row = singles.tile([1, 8 * 128], BF16)
                nc.sync.dma_start(gb_row, gb_d[:])
                ones_col = singles.tile([1, 128], BF16)
                nc.vector.memset(ones_col, 1.0)
                ones_sc = ones_col[:, :SC]

            # ---------------- phase A ----------------
            apool = ctx.enter_context(tc.tile_pool(name="apool", bufs=int(os.environ.get("APOOL", "4"))))
            with contextlib.ExitStack() as actx:
                apsum = actx.enter_context(
                    tc.tile_pool(name="apsum", bufs=1, space="PSUM"))

                # PE warmup: consume the weight-pack DMA on PE early
                # (borrows the ps_cn tag so phase A stays within 8 PSUM banks)
                ps_w1 = apsum.tile([128, 128], BF16, tag="ps_cn", bufs=2)
                nc.tensor.transpose(ps_w1, ident_bf, ident_bf)

                for s2 in range(S // 2):
                    s = 2 * s2
                    x_t = apool.tile([128, 2, T], BF16, tag="x_t", bufs=6)
                    nc.sync.dma_start_transpose(x_t[:, 0, :], x_d[s])
                    nc.sync.dma_start_transpose(x_t[:, 1, :], x_d[s + 1])

                    # conv_T = relu(W_c^T @ x_T): [F(2ch), seg, T]
                    ps_ct = apsum.tile([128, 2, 2, T], F32, tag="ps_ct",
                                       bufs=1)
                    for m in range(2):
                        nc.tensor.matmul(
                            ps_ct[:, m, :, :], cw_sb[:, bass.ts(m, 128)],
                            x_t, start=True, stop=True)
                    conv_t = apool.tile([128, 2, 2, T], BF16, tag="conv_t")
                    if zero_bias:
                        # single wide evac+relu amortizes the PSUM access
                        nc.vector.tensor_scalar_max(conv_t, ps_ct, 0.0)
                    else:
                        for mc in range(2):
                            nc.vector.tensor_scalar(
                                conv_t[:, mc, :, :], ps_ct[:, mc, :, :],
                                cb_sb[:, mc:mc + 1], 0.0, OP.add, OP.max)

                    conv_n = apool.tile([128, 2, 2, FF], BF16, tag="conv_n")
                    if os.environ.get("DMAT", "1") == "1":
                        # conv_N via SBUF->SBUF transpose-DMA: frees PE
                        # (8 transposes/pair) and DVE (the PSUM evac copy);
                        # the 16 DMA engines are otherwise idle here
                        for seg in range(2):
                            for tch in range(2):
                                for m in range(2):
                                    nc.sync.dma_start_transpose(
                                        conv_n[:, seg, tch, bass.ts(m, 128)],
                                        conv_t[:, m, seg, bass.ts(tch, 128)])
                    else:
                        # conv_N via PE transposes (bf16 psum pass-through)
                        ps_cn = apsum.tile([128, 2, 2, FF], BF16,
                                           tag="ps_cn", bufs=2)
                        for seg in range(2):
                            for tch in range(2):
                                for m in range(2):
                                    nc.tensor.transpose(
                                        ps_cn[:, seg, tch, bass.ts(m, 128)],
                                        conv_t[:, m, seg, bass.ts(tch, 128)],
                                        ident_bf)
                        nc.vector.tensor_copy(conv_n, ps_cn)

                    # scores = conv_N^T @ A (+ b): [seg, F-ch, T]
                    ps_s = apsum.tile([128, 2, 2, T], F32, tag="ps_s", bufs=2)
                    for seg in range(2):
                        for m in range(2):
                            for k in range(2):
                                nc.tensor.matmul(
                                    ps_s[:, seg, m, :],
                                    conv_n[:, seg, k, bass.ts(m, 128)],
                                    aw_sb[:, k, :],
                                    start=(k == 0),
                                    stop=(k == 1) and zero_bias)
                            if not zero_bias:
                                nc.tensor.matmul(
                                    ps_s[:, seg, m, :], ones_col, ab_row,
                                    start=False, stop=True)

                    if os.environ.get("EEPS", "0") == "1":
                        # exp writes PSUM (cheaper ACT access than SBUF);
                        # ec then runs on DVE (Pool cannot read PSUM)
                        ee = apsum.tile([128, 2, 2, T], BF16, tag="ps_cn",
                                        bufs=2)
                    else:
                        ee = apool.tile([128, 2, 2, T], BF16, tag="ee")
                    esum = apool.tile([128, 4], F32, tag="esum")
                    es4 = esum.rearrange("p (a b) -> p a b", a=2)
                    # ACT's accum_out costs an extra 187ns read-accumulator
                    # instruction per exp op; for alternate pairs compute the
                    # row-sums with one DVE reduce instead, splitting the
                    # load between the head's two busiest engines.
                    _esd = os.environ.get("ESUM_DVE", "1")
                    if _esd in ("0", "1"):
                        _ds = (s2 % 2 == int(_esd))
                    elif _esd == "23":
                        _ds = (s2 % 3 != 0)
                    elif _esd == "34":
                        _ds = (s2 % 4 != 0)
                    elif _esd == "13":
                        _ds = (s2 % 3 == 0)
                    else:
                        _ds = True
                    dve_sum = _ds and zero_bias
                    if dve_sum:
                        # no accumulator constraint -> ONE wide exp op
                        # (1.04us vs 4x0.5us: ACT pays a fixed ~285ns of
                        # access latency plus ~100ns gap per op)
                        nc.scalar.activation(ee, ps_s, AF.Exp)
                        nc.vector.tensor_reduce(
                            esum, ee, mybir.AxisListType.X, OP.add)
                    else:
                        for seg in range(2):
                            for m in range(2):
                                nc.scalar.activation(
                                    ee[:, seg, m, :], ps_s[:, seg, m, :],
                                    AF.Exp, accum_out=es4[:, seg, m:m + 1])
                    rinv = apool.tile([128, 4], F32, tag="rinv")
                    nc.vector.reciprocal(rinv, esum)
                    ri4 = rinv.rearrange("p (a b) -> p a b", a=2)

                    # x_att[:, m, s+seg, :] = E * rinv * conv_T  (packed T)
                    # split: ec = E*conv on Pool (TT), then *rinv on DVE (4x)
                    ec = apool.tile([128, 2, 2, T], BF16, tag="ec")
                    for seg in range(2):
                        if os.environ.get("EEPS", "0") == "1":
                            nc.vector.tensor_mul(
                                ec[:, seg, :, :], ee[:, seg, :, :],
                                conv_t[:, :, seg, :])
                        else:
                            nc.gpsimd.tensor_mul(
                                ec[:, seg, :, :], ee[:, seg, :, :],
                                conv_t[:, :, seg, :])
                    for seg in range(2):
                        for m in range(2):
                            nc.gpsimd.tensor_mul(
                                xatt[:, m, s + seg, :], ec[:, seg, m, :],
                                ri4[:, seg, m:m + 1].broadcast_to([128, T]))

            # ---------------- phase B: GRU over T steps, 2 chains ----------
            # gate columns in W/U: z=[0,256) m0,1 ; r=[256,512) m2,3 ;
            # h=[512,768) m4,5
            # psum tile layout [128, 8, SC]: z0 z1 r0 r1 | rh0 rh1 | xh0 xh1
            with contextlib.ExitStack() as bctx:
                hpool = bctx.enter_context(tc.tile_pool(name="hpool", bufs=2))
                gpool = bctx.enter_context(tc.tile_pool(name="gpool", bufs=3))
                bpsum = bctx.enter_context(
                    tc.tile_pool(name="bpsum", bufs=1, space="PSUM"))

                h_prev = [None, None]
                e_prev = [None, None]
                m1_prev = [None, None]
                rh_sbuf = [None, None]  # rh evacuated to SBUF (2x t1)
                pend = [None, None]  # (t, ps, rz, xh_sb) awaiting elementwise
                SPLIT_U = os.environ.get("SPLIT_U", "1") == "1"
                SPLIT_SIG = os.environ.get("SPLIT_SIG", "0") == "1"
                EVAC_T0 = int(os.environ.get("EVAC_T0", "0"))
                UZ_SPLIT = os.environ.get("UZ_SPLIT", "1") == "1"

                def emit_pe_act(c, t):
                    """Matmuls + sigmoid + xh evac for (chain c, step t)."""
                    cb = c * SC
                    ps = bpsum.tile([128, 8, SC], F32, tag=f"ps{c}", bufs=int(os.environ.get("PSB", "3")),
                                    name=f"ps{c}")
                    hp = h_prev[c]
                    ep = e_prev[c]
                    mp = m1_prev[c]

                    # x-part matmuls (independent of h)
                    zr_stop = (t == 0) and zero_bias
                    for j, m in enumerate((0, 1)):      # z gates
                        for k in range(2):
                            nc.tensor.matmul(
                                ps[:, j, :], wg_sb[:, k, bass.ts(m, 128)],
                                xatt[:, k, cb:cb + SC, t],
                                start=(k == 0), stop=(k == 1) and zr_stop,
                                skip_group_check=True)
                    for j, m in enumerate((2, 3)):      # r gates
                        for k in range(2):
                            nc.tensor.matmul(
                                ps[:, 2 + j, :],
                                wg_sb[:, k, bass.ts(m, 128)],
                                xatt[:, k, cb:cb + SC, t],
                                start=(k == 0), stop=(k == 1) and zr_stop,
                                skip_group_check=True)
                    for j, m in enumerate((4, 5)):      # h gate (xh)
                        for k in range(2):
                            nc.tensor.matmul(
                                ps[:, 6 + j, :],
                                wg_sb[:, k, bass.ts(m, 128)],
                                xatt[:, k, cb:cb + SC, t],
                                start=(k == 0),
                                stop=(k == 1) and zero_bias,
                                skip_group_check=True)

                    if not zero_bias:
                        # rank-1 bias adds; z0..r1 into [0:4],
                        # xh into [6:8], rh (br_h) into [4:6]
                        for j in range(4):
                            nc.tensor.matmul(
                                ps[:, j, :], gb_row[:, bass.ts(j, 128)],
                                ones_sc, start=False, stop=(t == 0),
                                skip_group_check=True)
                        for j in range(2):
                            nc.tensor.matmul(
                                ps[:, 6 + j, :],
                                gb_row[:, bass.ts(6 + j, 128)],
                                ones_sc, start=False, stop=True,
                                skip_group_check=True)
                        for j in range(2):
                            nc.tensor.matmul(
                                ps[:, 4 + j, :],
                                gb_row[:, bass.ts(4 + j, 128)],
                                ones_sc, start=True, stop=(t == 0),
                                skip_group_check=True)

                    if not SPLIT_SIG and t >= EVAC_T0:
                        # ACT evacuates xh early (depends only on Wx)
                        xh_sb = gpool.tile([128, 2, SC], BF16, tag=f"xh{c}",
                                           bufs=2, name=f"xh{c}")
                        nc.scalar.copy(xh_sb, ps[:, 6:8, :])
                    else:
                        xh_sb = None

                    rz = gpool.tile([128, 4, SC], BF16, tag=f"rz{c}",
                                    bufs=2, name=f"rz{c}")
                    if t > 0:
                        # U-part: r,z first (gate the sigmoid), then rh.
                        # With SPLIT_U, U_r and U_z use h = m1 + e by
                        # linearity: the m1-part operand landed ~500ns
                        # before e, so the sigmoid trigger is e, not h'.
                        if SPLIT_U and mp is not None:
                            for j, m in enumerate((2, 3)):
                                for k in range(2):
                                    nc.tensor.matmul(
                                        ps[:, 2 + j, :],
                                        wu_sb[:, k, bass.ts(m, 128)],
                                        mp[:, k, :],
                                        start=False, stop=False,
                                        skip_group_check=True)
                            if UZ_SPLIT:
                                for j, m in enumerate((0, 1)):
                                    for k in range(2):
                                        nc.tensor.matmul(
                                            ps[:, j, :],
                                            wu_sb[:, k, bass.ts(m, 128)],
                                            mp[:, k, :],
                                            start=False, stop=False,
                                            skip_group_check=True)
                        hsrc = ep if (SPLIT_U and mp is not None) else hp
                        zsrc = hsrc if UZ_SPLIT else hp
                        # k-outer: the k=0 mms consume e's first half, which
                        # the split e-STT produces ~100ns before the second
                        for k in range(2):
                            for j, m in enumerate((2, 3)):
                                nc.tensor.matmul(
                                    ps[:, 2 + j, :],
                                    wu_sb[:, k, bass.ts(m, 128)],
                                    hsrc[:, k, :],
                                    start=False, stop=(k == 1),
                                    skip_group_check=True)
                            for j, m in enumerate((0, 1)):
                                nc.tensor.matmul(
                                    ps[:, j, :],
                                    wu_sb[:, k, bass.ts(m, 128)],
                                    zsrc[:, k, :],
                                    start=False, stop=(k == 1),
                                    skip_group_check=True)
                        for j, m in enumerate((4, 5)):  # rh (always on h)
                            for k in range(2):
                                nc.tensor.matmul(
                                    ps[:, 4 + j, :],
                                    wu_sb[:, k, bass.ts(m, 128)],
                                    hp[:, k, :],
                                    start=(k == 0) and zero_bias,
                                    stop=(k == 1),
                                    skip_group_check=True)
                    if SPLIT_SIG:
                        # r first (on-path), z trails, on SEPARATE tiles so
                        # the r-path never waits on sig_z
                        rr = gpool.tile([128, 2, SC], BF16, tag=f"rr{c}",
                                        bufs=2, name=f"rr{c}")
                        ww = gpool.tile([128, 2, SC], BF16, tag=f"ww{c}",
                                        bufs=2, name=f"ww{c}")
                        nc.scalar.activation(rr, ps[:, 2:4, :], AF.Sigmoid)
                        nc.scalar.activation(ww, ps[:, 0:2, :], AF.Sigmoid)
                        pend[c] = (t, ps, (rr, ww), xh_sb)
                    else:
                        # sigmoid over [z;r] in one ACT op
                        nc.scalar.activation(rz, ps[:, 0:4, :], AF.Sigmoid)
                        pend[c] = (t, ps, rz, xh_sb)

                def emit_evac_rh(c):
                    """DVE-evacuate rh to SBUF during the sigmoid window:
                    t1 then runs in all-SBUF 2x mode (127 vs 258ns on the
                    critical path). Emitted after the OTHER chain's tail so
                    it never blocks ready work in the DVE queue."""
                    t, ps, _, _ = pend[c]
                    if t == 0 and zero_bias:
                        rh_sbuf[c] = None
                        return
                    rh_sb = gpool.tile([128, 2, SC], BF16, tag=f"rh{c}",
                                       bufs=2, name=f"rh{c}")
                    nc.vector.tensor_copy(rh_sb, ps[:, 4:6, :])
                    rh_sbuf[c] = rh_sb

                def emit_dve(c):
                    """Elementwise chain for the pending (chain c) step.
                    The d/e blend ops run on Pool to cut DVE occupancy."""
                    t, ps, rz, xh_sb = pend[c]
                    hp = h_prev[c]
                    if SPLIT_SIG:
                        rr_t, ww_t = rz
                    else:
                        rr_t, ww_t = rz[:, 2:4, :], rz[:, 0:2, :]
                    h_new = hpool.tile([128, 2, SC], BF16, tag=f"h{c}",
                                       name=f"h{c}")
                    hh = gpool.tile([128, 2, SC], BF16, tag=f"hh{c}",
                                    bufs=2, name=f"hh{c}")
                    e = gpool.tile([128, 2, SC], BF16, tag=f"e{c}",
                                   bufs=3, name=f"e{c}")
                    have_rh = (t > 0) or not zero_bias
                    # ww is already w = 1-z (z weights negated on host).
                    # Off-chain on Pool: m1 = z*h = h - w*h
                    w = ww_t
                    if t > 0:
                        mw = gpool.tile([128, 2, SC], BF16, tag=f"mw{c}",
                                        bufs=2, name=f"mw{c}")
                        m1 = gpool.tile([128, 2, SC], BF16, tag=f"m1{c}",
                                        bufs=3, name=f"m1{c}")
                        nc.gpsimd.tensor_mul(mw, w, hp)
                        nc.gpsimd.tensor_sub(m1, hp, mw)
                    if have_rh:
                        t1 = gpool.tile([128, 2, SC], BF16, tag=f"t1{c}",
                                        bufs=2, name=f"t1{c}")
                        q = gpool.tile([128, 2, SC], BF16, tag=f"q{c}",
                                       bufs=2, name=f"q{c}")
                        rh_src = (rh_sbuf[c] if rh_sbuf[c] is not None
                                  else ps[:, 4:6, :])
                        nc.vector.tensor_mul(t1, rr_t, rh_src)
                        nc.vector.tensor_add(
                            q, t1,
                            xh_sb if xh_sb is not None else ps[:, 6:8, :])
                    else:
                        q = xh_sb if xh_sb is not None else ps[:, 6:8, :]
                    # e = relu(q)*w fused in ONE DVE STT op (the r-path
                    # terminus); h' = e + m1 runs off-path on Pool
                    if os.environ.get("FUSE_E", "1") == "1":
                        if os.environ.get("ESPLIT", "0") == "1":
                            # two half-STTs: e[k0] lands first so the next
                            # step's k=0 U matmuls start earlier
                            nc.vector.scalar_tensor_tensor(
                                e[:, 0, :], q[:, 0, :], 0.0, w[:, 0, :],
                                OP.max, OP.mult)
                            nc.vector.scalar_tensor_tensor(
                                e[:, 1, :], q[:, 1, :], 0.0, w[:, 1, :],
                                OP.max, OP.mult)
                        else:
                            nc.vector.scalar_tensor_tensor(
                                e, q, 0.0, w, OP.max, OP.mult)
                    else:
                        nc.vector.tensor_scalar_max(hh, q, 0.0)
                        nc.gpsimd.tensor_mul(e, w, hh)
                    if t > 0:
                        nc.gpsimd.tensor_add(h_new, e, m1)
                        h_prev[c] = h_new
                        m1_prev[c] = m1
                    else:
                        h_prev[c] = e                          # h0 = e0
                        m1_prev[c] = None
                    e_prev[c] = e

                RHEVAC = os.environ.get("RHEVAC", "0") == "1"
                for t in range(T):
                    emit_pe_act(0, t)
                    if t > 0:
                        emit_dve(1)
                    if RHEVAC:
                        emit_evac_rh(0)
                    emit_pe_act(1, t)
                    emit_dve(0)
                    if RHEVAC:
                        emit_evac_rh(1)
                emit_dve(1)

                # output: transpose h back to [S, H] and store fp32
                ps_o = bpsum.tile([64, 2, 2, 128], BF16, tag="ps_o", bufs=1)
                for c in range(2):
                    for ch in range(2):
                        nc.tensor.transpose(
                            ps_o[:, c, ch, :], h_prev[c][:, ch, :], ident_bf)
                out_sb = gpool.tile([64, 2, 2, 128], F32, tag="out_sb")
                nc.vector.tensor_copy(out_sb, ps_o)
                for c in range(2):
                    nc.sync.dma_start(
                        out_d[c * SC:(c + 1) * SC].rearrange(
                            "s (ch p) -> s ch p", ch=2), out_sb[:, c])

    _split_multi_waits(nc)
    return nc


def _split_multi_waits(nc: bass.Bass):
    """Encode at most ONE semaphore wait per ISA instruction: hoist extras
    onto preceding same-engine NoOp carriers."""
    fn = nc.m.functions[0]
    for blk in fn.blocks:
        insts = list(blk.instructions)
        out = []
        changed = False
        for inst in insts:
            si = inst.sync_info
            waits = list(si.on_wait) if si is not None else []
            if len(waits) > 1:
                changed = True
                for w in waits[:-1]:
                    out.append(mybir.InstNoOp(
                        name=f"I-wsplit-{nc.next_id()}",
                        engine=inst.engine,
                        ins=[], outs=[],
                        sync_info=mybir.SyncInfo(on_wait=[w], on_update=[]),
                    ))
                inst.sync_info = mybir.SyncInfo(
                    on_wait=[waits[-1]], on_update=list(si.on_update))
            out.append(inst)
        if changed:
            blk.instructions = out


_CACHE = {}


def _get_nc(zero_bias: bool) -> bass.Bass:
    if zero_bias not in _CACHE:
        _CACHE[zero_bias] = build(zero_bias)
    return _CACHE[zero_bias]


def _pack_weights(conv_w, attn_w, gru_w, gru_u):
    bf = ml_dtypes.bfloat16
    # z-gate columns negated: sigmoid of the negated preact yields w = 1-z
    gru_w = gru_w.copy(); gru_w[:, :256] *= -1.0
    gru_u = gru_u.copy(); gru_u[:, :256] *= -1.0
    cw = (conv_w[0] if conv_w.ndim == 3 else conv_w).astype(bf)  # [128, 256]
    aw = attn_w.astype(bf).reshape(2, 128, T).transpose(1, 0, 2).reshape(
        128, 2 * T)
    wg = gru_w.astype(bf).reshape(2, 128, 768).transpose(1, 0, 2).reshape(
        128, 1536)
    wu = gru_u.astype(bf).reshape(2, 128, 768).transpose(1, 0, 2).reshape(
        128, 1536)
    ident = np.eye(128, dtype=np.float32).astype(bf)
    return np.ascontiguousarray(
        np.concatenate([cw, aw, wg, wu, ident], axis=1), bf)


def kernel(x, conv_w, conv_b, attn_w, attn_b, gru_w, gru_u, gru_b):
    x = np.asarray(x, dtype=np.float32)
    conv_w = np.asarray(conv_w, dtype=np.float32)
    conv_b = np.asarray(conv_b, dtype=np.float32)
    attn_w = np.asarray(attn_w, dtype=np.float32)
    attn_b = np.asarray(attn_b, dtype=np.float32)
    gru_w = np.asarray(gru_w, dtype=np.float32)
    gru_u = np.asarray(gru_u, dtype=np.float32)
    gru_b = np.asarray(gru_b, dtype=np.float32)

    zero_bias = (
        not conv_b.any() and not attn_b.any() and not gru_b.any())

    nc = _get_nc(zero_bias)

    xs_bf = x.reshape(B * LTMS, T, C_IN).astype(ml_dtypes.bfloat16)
    bfpack = _pack_weights(conv_w, attn_w, gru_w, gru_u)

    in_maps = []
    for c in range(NCORES):
        m = {
            "x_shard": np.ascontiguousarray(xs_bf[c * S: (c + 1) * S]),
            "bfpack": bfpack,
        }
        if not zero_bias:
            bi, br = gru_b[0], gru_b[1]
            comb = bi + br
            gbr = np.zeros((1, 8 * 128), np.float32)
            gbr[0, 0:512] = comb[0:512]          # z0 z1 r0 r1
            gbr[0, 0:256] *= -1.0                # negated z preact -> w
            gbr[0, 512:768] = br[512:768]        # rh0 rh1
            gbr[0, 768:1024] = bi[512:768]       # xh0 xh1
            m["conv_b2"] = np.ascontiguousarray(
                conv_b.reshape(2, 128).T, np.float32)
            m["attn_b"] = attn_b.reshape(1, T).astype(ml_dtypes.bfloat16)
            m["gbias_row"] = gbr.astype(ml_dtypes.bfloat16)
        in_maps.append(m)

    res = run_bass_kernel_spmd(nc, in_maps, core_ids=list(range(NCORES)))
    outs = [res.results[c]["h_out"] for c in range(NCORES)]
    h = np.concatenate(outs, axis=0)  # [1024, 256]
    return h.reshape(B, LTMS, HH).astype(np.float32)


if __name__ == "__main__":
    nc = _get_nc(True)
    print("built ok")

